# revision 1
# baseline (speedup 1.0000x reference)
"""Trainium2 Bass kernel for nn_AttentionBlock (GroupNorm + MHA + proj + residual).

Input  x: [16, 512, 32, 32] fp32.  8 NeuronCores, data-parallel over batch
(2 images per core).  Everything is hardcoded for these shapes.

Per-image dataflow on one core (channel-major layout [c, n], n = h*w = 1024):
  1. GroupNorm(8 groups of 64 channels): per-channel mean/var via bn_stats,
     group aggregation via a tiny PE matmul with a group-selector matrix,
     per-channel scale/shift broadcast back with small DMAs, applied on DVE.
  2. QKV 1x1 conv as matmuls.  q,k produced channel-major [o, n]; v produced
     PRE-TRANSPOSED [n, c_v] by using xn as the stationary operand — so the
     attention stage needs no PE transposes at all.
  3. Attention per head (dh=128 = exactly one partition tile):
       S^T[m,n] = K^T Q      (lhsT = K-slice, rhs = Q)
       P^T      = exp(scale * S^T)   on ScalarE straight out of PSUM
       rowsum[n]= ones^T @ P^T        accumulated on PE
       outT[d,n]= (V^T-slice)^T @ P^T accumulated over m-tiles
     PV psums are evacuated unnormalized (DVE copy) so the banks free
     immediately; softmax normalization (a per-column scale by 1/rowsum)
     runs off the critical path: fast-approx reciprocal on DVE, GpSimd
     partition-broadcast, in-place column scale of the output tile.
     No max-subtraction: scores are ~N(0,1), exp is safe in fp32.
  4. proj matmul; bias + residual fused into one scalar_tensor_tensor per tile.

The two images are software-pipelined at stage level
  gn0 qkv0 gn1 attn0 qkv1 [attn1 heads x proj0 chunks zipper] proj1
with v/proj weight DMAs deferred past the startup x+wqk load burst, so the
PE never waits on the serial GN chain, an attention tail, or DMA bandwidth.

Matmul operands are stored as float32r (full-rate fp32 on the PE for moving
dim >= 256, ~tf32 accuracy).  Set MM_MODE="bf16" for the bf16 variant.
"""

import os
import numpy as np

import concourse.bass as bass
import concourse.bacc as bacc
import concourse.tile as tile
import concourse.bass_isa as bass_isa
from concourse import mybir
from concourse.bass_utils import run_bass_kernel_spmd

N_CORES = 8
B, C, HH, WW = 16, 512, 32, 32
N = HH * WW            # 1024 tokens per image
NH, DH = 4, 128        # heads, head dim
G, GS = 8, 64          # groups, channels per group
B_LOC = B // N_CORES   # images per core
EPS = 1e-5
CT = C // 128          # 4 channel tiles
NT = N // 128          # 8 token tiles
NCH = N // 512         # 2 free-dim chunks of 512
SCALE = float(DH) ** -0.5

MM_MODE = os.environ.get("MM_MODE", "f32r")   # "f32r" | "bf16" | "f32"

f32 = mybir.dt.float32
if MM_MODE == "bf16":
    MD = mybir.dt.bfloat16
elif MM_MODE == "f32":
    MD = f32
else:
    MD = mybir.dt.float32r

AF = mybir.ActivationFunctionType
OP = mybir.AluOpType


def _bcast_rows(dst_ap, src_row_ap, reps):
    """DMA-broadcast a [1, F] SBUF row to [reps, F] partitions."""
    src = bass.AP(
        tensor=src_row_ap.tensor,
        offset=src_row_ap.offset,
        ap=[[1, 1], [0, reps]] + [[d[0], d[1]] for d in src_row_ap.ap[1:]],
    )
    return src, dst_ap


def build_program():
    nc = bacc.Bacc("TRN2", target_bir_lowering=False, debug=False)

    x_d = nc.dram_tensor("x", [B_LOC, C, N], f32, kind="ExternalInput").ap()
    wqk_d = nc.dram_tensor("wqkT", [C, 2 * C], MD, kind="ExternalInput").ap()
    wv_d = nc.dram_tensor("wvT", [C, C], MD, kind="ExternalInput").ap()
    wp_d = nc.dram_tensor("wpT", [C, C], MD, kind="ExternalInput").ap()
    qkb_d = nc.dram_tensor("qkb", [2 * C], f32, kind="ExternalInput").ap()
    vb_d = nc.dram_tensor("vb", [C], f32, kind="ExternalInput").ap()
    pb_d = nc.dram_tensor("pb", [C], f32, kind="ExternalInput").ap()
    gam_d = nc.dram_tensor("gamma", [C], f32, kind="ExternalInput").ap()
    bet_d = nc.dram_tensor("beta", [C], f32, kind="ExternalInput").ap()
    out_d = nc.dram_tensor("out", [B_LOC, C, N], f32, kind="ExternalOutput").ap()

    with tile.TileContext(nc) as tc:
        with (
            tc.tile_pool(name="wpool", bufs=1) as wpool,
            tc.tile_pool(name="xpool", bufs=1) as xpool,
            tc.tile_pool(name="xnpool", bufs=1) as xnpool,
            tc.tile_pool(name="qkpool", bufs=1) as qkpool,
            tc.tile_pool(name="vtpool", bufs=1) as vtpool,
            tc.tile_pool(name="ptpool", bufs=3) as ptpool,
            tc.tile_pool(name="otpool", bufs=2) as otpool,
            tc.tile_pool(name="rpool", bufs=2) as rpool,
            tc.tile_pool(name="outpool", bufs=2) as outpool,
            tc.tile_pool(name="rxpool", bufs=2) as rxpool,
            tc.tile_pool(name="spool", bufs=2) as spool,
            tc.tile_pool(name="chpool", bufs=4) as chpool,
            tc.tile_pool(name="mmps", bufs=4, space="PSUM") as mmps,
            tc.tile_pool(name="accps", bufs=2, space="PSUM") as accps,
        ):
            # ---- prefetch x for all images (overlaps weight DMAs) ----
            xts = []
            for img in range(B_LOC):
                xt = xpool.tile([128, CT, N], f32, tag="x", name=f"xt{img}")
                xr = x_d[img].rearrange("(t p) n -> p t n", p=128)
                for ct in range(CT):
                    nc.sync.dma_start(xt[:, ct, :], xr[:, ct, :])
                xts.append(xt)

            # ---- persistent weights / constants ----
            wqk_sb, wv_sb, wp_sb = [], [], []
            for kt in range(CT):
                t = wpool.tile([128, 2 * C], MD, tag=f"wqk{kt}", name=f"wqk{kt}")
                nc.sync.dma_start(t[:], wqk_d[kt * 128:(kt + 1) * 128, :])
                wqk_sb.append(t)

            def load_late_weights():
                for kt in range(CT):
                    t = wpool.tile([128, C], MD, tag=f"wv{kt}", name=f"wv{kt}")
                    nc.sync.dma_start(t[:], wv_d[kt * 128:(kt + 1) * 128, :])
                    wv_sb.append(t)
                    t = wpool.tile([128, C], MD, tag=f"wp{kt}", name=f"wp{kt}")
                    nc.sync.dma_start(t[:], wp_d[kt * 128:(kt + 1) * 128, :])
                    wp_sb.append(t)

            qkb_sb = wpool.tile([128, 2 * C // 128], f32, tag="qkb")
            nc.sync.dma_start(qkb_sb[:], qkb_d.rearrange("(t p) -> p t", p=128))
            pb_sb = wpool.tile([128, CT], f32, tag="pb")
            nc.sync.dma_start(pb_sb[:], pb_d.rearrange("(t p) -> p t", p=128))
            gam_sb = wpool.tile([128, CT], f32, tag="gam")
            nc.sync.dma_start(gam_sb[:], gam_d.rearrange("(t p) -> p t", p=128))
            bet_sb = wpool.tile([128, CT], f32, tag="bet")
            nc.sync.dma_start(bet_sb[:], bet_d.rearrange("(t p) -> p t", p=128))
            vb_bc = wpool.tile([128, C], f32, tag="vbbc")
            nc.sync.dma_start(
                vb_bc[:],
                bass.AP(tensor=vb_d.tensor, offset=vb_d.offset,
                        ap=[[0, 128], [1, C]]))

            sel = wpool.tile([128, 2], f32, tag="sel")
            nc.vector.memset(sel[0:64, 0:1], 1.0 / GS)
            nc.vector.memset(sel[64:128, 0:1], 0.0)
            nc.vector.memset(sel[0:64, 1:2], 0.0)
            nc.vector.memset(sel[64:128, 1:2], 1.0 / GS)
            ones_f = wpool.tile([128, 1], f32, tag="onesf")
            nc.vector.memset(ones_f[:], 1.0)
            ones_t = wpool.tile([128, 1], MD, tag="ones")
            nc.vector.tensor_copy(ones_t[:], ones_f[:])
            eps_t = wpool.tile([2, 1], f32, tag="eps")
            nc.vector.memset(eps_t[:], EPS)

            def stage_gn(img):
                """GroupNorm: returns normalized xn tile (matmul dtype)."""
                xt = xts[img]
                stats2 = spool.tile([128, 2 * CT], f32, tag="stats2",
                                    name=f"stats2_{img}")
                for ct in range(CT):
                    st = spool.tile([128, 2, 6], f32, tag="bnst", name="st")
                    nc.vector.bn_stats(st[:, 0, :], xt[:, ct, 0:512])
                    nc.vector.bn_stats(st[:, 1, :], xt[:, ct, 512:1024])
                    mv = spool.tile([128, 2], f32, tag="mv", name="mv")
                    nc.vector.bn_aggr(mv[:], st[:])
                    nc.vector.tensor_copy(stats2[:, 2 * ct:2 * ct + 1], mv[:, 0:1])
                    nc.vector.tensor_mul(
                        stats2[:, 2 * ct + 1:2 * ct + 2], mv[:, 0:1], mv[:, 0:1])
                    nc.vector.tensor_add(
                        stats2[:, 2 * ct + 1:2 * ct + 2],
                        stats2[:, 2 * ct + 1:2 * ct + 2], mv[:, 1:2])
                # aggregate per group across the 64 partitions of each group
                psg = mmps.tile([2, 2 * CT], f32, tag="mm", name="psg")
                nc.tensor.matmul(psg[:], sel[:], stats2[:], start=True, stop=True)
                gs = spool.tile([2, 2 * CT], f32, tag="gs", name="gs")
                nc.scalar.activation(gs[:], psg[:], AF.Copy)
                gs3 = gs[:].rearrange("p (t s) -> p t s", s=2)
                # var = E2 - mu^2 ; rstd = 1/sqrt(var+eps)
                tmp = spool.tile([2, CT], f32, tag="gtmp", name="tmp")
                nc.vector.tensor_mul(tmp[:], gs3[:, :, 0], gs3[:, :, 0])
                var_g = spool.tile([2, CT], f32, tag="gvar", name="var_g")
                nc.vector.tensor_sub(var_g[:], gs3[:, :, 1], tmp[:])
                sd = spool.tile([2, CT], f32, tag="gsd", name="sd")
                nc.scalar.activation(sd[:], var_g[:], AF.Sqrt, bias=eps_t[:])
                rstd_g = spool.tile([2, CT], f32, tag="grstd", name="rstd_g")
                rscr = spool.tile([2, CT], f32, tag="grscr", name="rscr")
                nc.vector.reciprocal_approx_accurate(rstd_g[:], sd[:], rscr[:])

                xnt = xnpool.tile([128, CT, N], MD, tag="xn", name=f"xn{img}")
                for ct in range(CT):
                    mu_ch = chpool.tile([128, 1], f32, tag="much", name="mu_ch")
                    sg = gs[:, 2 * ct:2 * ct + 1]
                    nc.sync.dma_start(
                        mu_ch[:],
                        bass.AP(tensor=sg.tensor, offset=sg.offset,
                                ap=[[sg.ap[0][0], 2], [0, GS]]))
                    rs_ch = chpool.tile([128, 1], f32, tag="rsch", name="rs_ch")
                    sg = rstd_g[:, ct:ct + 1]
                    nc.sync.dma_start(
                        rs_ch[:],
                        bass.AP(tensor=sg.tensor, offset=sg.offset,
                                ap=[[sg.ap[0][0], 2], [0, GS]]))
                    a_ch = chpool.tile([128, 1], f32, tag="ach", name="a_ch")
                    nc.vector.tensor_mul(a_ch[:], rs_ch[:], gam_sb[:, ct:ct + 1])
                    b_ch = chpool.tile([128, 1], f32, tag="bch", name="b_ch")
                    nc.vector.tensor_mul(b_ch[:], mu_ch[:], a_ch[:])
                    nc.vector.tensor_sub(b_ch[:], bet_sb[:, ct:ct + 1], b_ch[:])
                    nc.vector.tensor_scalar(
                        out=xnt[:, ct, :], in0=xt[:, ct, :],
                        scalar1=a_ch[:], scalar2=b_ch[:],
                        op0=OP.mult, op1=OP.add)
                return xnt

            def stage_qkv(img, xnt):
                """q,k channel-major [o, n] and v pre-transposed [n, c_v]."""
                qk = qkpool.tile([128, 2 * CT, N], MD, tag="qk", name=f"qk{img}")
                for mt in range(2 * CT):
                    pss = [mmps.tile([128, 512], f32, tag="mm", name=f"pss{ch}")
                           for ch in range(NCH)]
                    for kt in range(CT):
                        for ch in range(NCH):
                            nc.tensor.matmul(
                                pss[ch][:],
                                wqk_sb[kt][:, mt * 128:(mt + 1) * 128],
                                xnt[:, kt, ch * 512:(ch + 1) * 512],
                                start=(kt == 0), stop=(kt == CT - 1))
                    for ch in range(NCH):
                        nc.scalar.activation(
                            qk[:, mt, ch * 512:(ch + 1) * 512], pss[ch][:],
                            AF.Identity, bias=qkb_sb[:, mt:mt + 1])

                vt = vtpool.tile([128, NT, C], MD, tag="vt", name=f"vt{img}")
                for nt in range(NT):
                    ps = mmps.tile([128, 512], f32, tag="mm", name="ps_v")
                    for kt in range(CT):
                        nc.tensor.matmul(
                            ps[:],
                            xnt[:, kt, nt * 128:(nt + 1) * 128],
                            wv_sb[kt][:],
                            start=(kt == 0), stop=(kt == CT - 1))
                    nc.vector.tensor_add(vt[:, nt, :], ps[:], vb_bc[:])
                return qk, vt

            def stage_attn(img, qk, vt, after_head=None):
                """S^T = K^T Q, P^T = exp, PV + rowsum on PE, normalize late."""
                ot = otpool.tile([128, NH, N], MD, tag="ot", name=f"ot{img}")
                for h in range(NH):
                    ps_o = [accps.tile([128, 512], f32, tag="acc", name=f"ps_o{ch}")
                            for ch in range(NCH)]
                    ps_r = accps.tile([1, N], f32, tag="accr", bufs=1, name="ps_r")
                    for mt in range(NT):
                        pt = ptpool.tile([128, N], MD, tag="pt", name="pt")
                        for ch in range(NCH):
                            ps_s = mmps.tile([128, 512], f32, tag="mm", name="ps_s")
                            nc.tensor.matmul(
                                ps_s[:],
                                qk[:, NH + h, mt * 128:(mt + 1) * 128],
                                qk[:, h, ch * 512:(ch + 1) * 512],
                                start=True, stop=True)
                            nc.scalar.activation(
                                pt[:, ch * 512:(ch + 1) * 512], ps_s[:],
                                AF.Exp, scale=SCALE)
                        for ch in range(NCH):
                            nc.tensor.matmul(
                                ps_o[ch][:],
                                vt[:, mt, h * 128:(h + 1) * 128],
                                pt[:, ch * 512:(ch + 1) * 512],
                                start=(mt == 0), stop=(mt == NT - 1))
                        for ch in range(NCH):
                            nc.tensor.matmul(
                                ps_r[:, ch * 512:(ch + 1) * 512],
                                ones_t[:],
                                pt[:, ch * 512:(ch + 1) * 512],
                                start=(mt == 0), stop=(mt == NT - 1))
                    # evacuate PV psums immediately (frees banks for next head)
                    for ch in range(NCH):
                        nc.vector.tensor_copy(
                            ot[:, h, ch * 512:(ch + 1) * 512], ps_o[ch][:])
                    # normalization off the critical path: reciprocal of the
                    # rowsums, GpSimd partition-broadcast, column-scale ot.
                    rb = rpool.tile([128, N], f32, tag="rb", name="rb")
                    for ch in range(NCH):
                        rinv = rpool.tile([1, 512], f32, tag="rinv",
                                          name=f"rinv{ch}", bufs=4)
                        nc.vector.reciprocal_approx_fast(
                            rinv[:], ps_r[:, ch * 512:(ch + 1) * 512])
                        nc.gpsimd.partition_broadcast(
                            rb[:, ch * 512:(ch + 1) * 512], rinv[:],
                            channels=128)
                    for ch in range(NCH):
                        nc.vector.tensor_mul(
                            ot[:, h, ch * 512:(ch + 1) * 512],
                            ot[:, h, ch * 512:(ch + 1) * 512],
                            rb[:, ch * 512:(ch + 1) * 512])
                    if after_head is not None:
                        after_head(h)
                return ot

            def emit_proj_chunk(img, ot, pt_i):
                if img == B_LOC - 1:
                    rx = xts[img][:, pt_i, :]
                else:
                    rx = rxpool.tile([128, N], f32, tag="rx",
                                     name=f"rx{img}_{pt_i}")
                    nc.sync.dma_start(
                        rx[:], x_d[img, pt_i * 128:(pt_i + 1) * 128, :])
                pss = [mmps.tile([128, 512], f32, tag="mm", name=f"psp{ch}")
                       for ch in range(NCH)]
                for kt in range(CT):
                    for ch in range(NCH):
                        nc.tensor.matmul(
                            pss[ch][:],
                            wp_sb[kt][:, pt_i * 128:(pt_i + 1) * 128],
                            ot[:, kt, ch * 512:(ch + 1) * 512],
                            start=(kt == 0), stop=(kt == CT - 1))
                outt = outpool.tile([128, N], f32, tag="outt",
                                    name=f"o{img}_{pt_i}")
                for ch in range(NCH):
                    nc.vector.scalar_tensor_tensor(
                        out=outt[:, ch * 512:(ch + 1) * 512],
                        in0=pss[ch][:],
                        scalar=pb_sb[:, pt_i:pt_i + 1],
                        in1=rx[:, ch * 512:(ch + 1) * 512],
                        op0=OP.add, op1=OP.add)
                nc.sync.dma_start(
                    out_d[img, pt_i * 128:(pt_i + 1) * 128, :], outt[:])

            def stage_proj(img, ot):
                for pt_i in range(CT):
                    emit_proj_chunk(img, ot, pt_i)

            # ---- software pipeline over the two images ----
            xn0 = stage_gn(0)
            load_late_weights()
            qk0, vt0 = stage_qkv(0, xn0)
            xn1 = stage_gn(1)
            ot0 = stage_attn(0, qk0, vt0)
            qk1, vt1 = stage_qkv(1, xn1)
            ot1 = stage_attn(1, qk1, vt1,
                             after_head=lambda h: emit_proj_chunk(0, ot0, h))
            stage_proj(1, ot1)

    nc.compile()
    return nc


_NC_CACHE = None


def _get_nc():
    global _NC_CACHE
    if _NC_CACHE is None:
        _NC_CACHE = build_program()
    return _NC_CACHE


def _host_prep(x, norm_gamma, norm_beta, qkv_w, qkv_b, proj_w, proj_b):
    def cvt(a):
        a = np.ascontiguousarray(a, dtype=np.float32)
        if MM_MODE == "bf16":
            import ml_dtypes
            return a.astype(ml_dtypes.bfloat16)
        return a
    common = {
        "wqkT": cvt(qkv_w[:2 * C].T),
        "wvT": cvt(qkv_w[2 * C:].T),
        "wpT": cvt(proj_w.T),
        "qkb": np.ascontiguousarray(qkv_b[:2 * C], dtype=np.float32),
        "vb": np.ascontiguousarray(qkv_b[2 * C:], dtype=np.float32),
        "pb": np.ascontiguousarray(proj_b, dtype=np.float32),
        "gamma": np.ascontiguousarray(norm_gamma, dtype=np.float32),
        "beta": np.ascontiguousarray(norm_beta, dtype=np.float32),
    }
    xr = np.ascontiguousarray(np.asarray(x, dtype=np.float32).reshape(B, C, N))
    in_maps = []
    for c in range(N_CORES):
        m = dict(common)
        m["x"] = np.ascontiguousarray(xr[c * B_LOC:(c + 1) * B_LOC])
        in_maps.append(m)
    return in_maps


def run(inputs, trace=False):
    nc = _get_nc()
    in_maps = _host_prep(**inputs)
    res = None
    for attempt in range(3):
        try:
            res = run_bass_kernel_spmd(
                nc, in_maps, core_ids=list(range(N_CORES)), trace=trace)
            break
        except Exception:
            # rare transient NRT_EXEC_UNIT_UNRECOVERABLE on a cold device;
            # a re-run on the recovered device succeeds.
            if attempt == 2:
                raise
    parts = [res.results[c]["out"] for c in range(N_CORES)]
    out = np.concatenate(parts, axis=0).reshape(B, C, HH, WW)
    return out.astype(np.float32), res


def kernel(**inputs):
    out, _ = run(inputs, trace=False)
    return out



# revision 10
# speedup vs baseline: 1.0804x; 1.0804x over previous
"""Trainium2 Bass kernel for nn_AttentionBlock (GroupNorm + MHA + proj + residual).

Input  x: [16, 512, 32, 32] fp32.  8 NeuronCores, data-parallel over batch
(2 images per core).  Everything is hardcoded for these shapes.

fp8-e4m3 DoubleRow edition.  All matmuls except the score matmul run in
fp8 e4m3 with perf_mode=DoubleRow (K=256 per instruction, 0.5 cyc/col):
  - QKV and proj weights are host-prescaled by 16 (fp8-friendly range)
    and unscaled in the psum evacuation.
  - q,k are evacuated to bf16; the S^T = K^T Q matmul runs in bf16
    (full-rate, contraction d=128 can't double-row without a partition
    remap).
  - exp runs on the Scalar engine with bias -ln(4) so P/4 stays inside
    e4m3 range (max ~240); the /4 cancels in the softmax ratio because
    the SAME fp8 pt tensor feeds both the PV matmul and the rowsum.
  - rowsum via a skinny ones-lhsT DoubleRow matmul (out [1, n]); the
    reciprocal is broadcast to 128 partitions on GpSimd.
  - v is computed pre-transposed [m, c_v] as fp8 with mt-pair planes so
    PV contracts 256 tokens per instruction.
Engine split: exp + q-evac on Scalar; k-evac, v-evac, reciprocal,
ot-normalize, proj-unscale on DVE; GroupNorm apply, rowsum broadcast and
the residual add on GpSimd (no PSUM port there, SBUF-only work).
DMA: x + out on the Act HWDGE queue, weights + stores on the SP queue.

Numerics: scores are ~N(0,1) over 1024 keys => near-flat softmax, so fp8
error in q/k/v/xn washes out in the weighted average; tolerance is 2e-2
and this lands ~1e-3.
"""

import math
import numpy as np

import concourse.bass as bass
import concourse.bacc as bacc
import concourse.tile as tile
from concourse import mybir
from concourse.bass_utils import run_bass_kernel_spmd

N_CORES = 8
B, C, HH, WW = 16, 512, 32, 32
N = HH * WW            # 1024 tokens per image
NH, DH = 4, 128        # heads, head dim
G, GS = 8, 64          # groups, channels per group
B_LOC = B // N_CORES   # images per core
EPS = 1e-5
CT = C // 128          # 4 channel tiles
NT = N // 128          # 8 token tiles
SCALE = float(DH) ** -0.5
WS = 16.0              # host-side weight prescale
EB = -math.log(4.0)    # exp bias: pt = P/4, keeps e4m3 in range
OS = 1.0 / 16.0        # ones value: rowsum psum = sum(pt)/16

f32 = mybir.dt.float32
bf16 = mybir.dt.bfloat16
fp8 = mybir.dt.float8e4
AF = mybir.ActivationFunctionType
OP = mybir.AluOpType
DR = mybir.MatmulPerfMode.DoubleRow


def build_program():
    nc = bacc.Bacc("TRN2", target_bir_lowering=False, debug=False)

    x_d = nc.dram_tensor("x", [B_LOC, C, N], f32, kind="ExternalInput").ap()
    wqk_d = nc.dram_tensor("wqk", [128, 2, 2, 2 * C], fp8, kind="ExternalInput").ap()
    wv_d = nc.dram_tensor("wv", [128, 2, 2, C], fp8, kind="ExternalInput").ap()
    wp_d = nc.dram_tensor("wp", [128, 2, 2, C], fp8, kind="ExternalInput").ap()
    qkb_d = nc.dram_tensor("qkb", [128, 2 * C // 128], f32, kind="ExternalInput").ap()
    vb_d = nc.dram_tensor("vb", [C], f32, kind="ExternalInput").ap()
    pb_d = nc.dram_tensor("pb", [128, CT], f32, kind="ExternalInput").ap()
    gam_d = nc.dram_tensor("gamma", [128, CT], f32, kind="ExternalInput").ap()
    bet_d = nc.dram_tensor("beta", [128, CT], f32, kind="ExternalInput").ap()
    out_d = nc.dram_tensor("out", [B_LOC, C, N], f32, kind="ExternalOutput").ap()

    with tile.TileContext(nc) as tc:
        with (
            tc.tile_pool(name="wpool", bufs=1) as wpool,
            tc.tile_pool(name="xpool", bufs=2) as xpool,
            tc.tile_pool(name="xnpool", bufs=2) as xnpool,
            tc.tile_pool(name="qkpool", bufs=2) as qkpool,
            tc.tile_pool(name="vtpool", bufs=2) as vtpool,
            tc.tile_pool(name="ptpool", bufs=2) as ptpool,
            tc.tile_pool(name="otpool", bufs=2) as otpool,
            tc.tile_pool(name="rpool", bufs=2) as rpool,
            tc.tile_pool(name="tpool", bufs=2) as tpool,
            tc.tile_pool(name="outpool", bufs=2) as outpool,
            tc.tile_pool(name="spool", bufs=2) as spool,
            tc.tile_pool(name="chpool", bufs=4) as chpool,
            tc.tile_pool(name="mmps", bufs=2, space="PSUM") as mmps,
            tc.tile_pool(name="pvps", bufs=1, space="PSUM") as pvps,
            tc.tile_pool(name="rsps", bufs=1, space="PSUM") as rsps,
        ):
            # ---- x on the Act HWDGE queue; weights on the SP queue ----
            xts = []
            for img in range(B_LOC):
                xt = xpool.tile([128, CT, N], f32, tag="x", name=f"xt{img}")
                xr = x_d[img].rearrange("(t p) n -> p t n", p=128)
                for ct in range(CT):
                    nc.scalar.dma_start(xt[:, ct, :], xr[:, ct, :])
                xts.append(xt)

            wqk_sb = wpool.tile([128, 2, 2, 2 * C], fp8, tag="wqk")
            nc.sync.dma_start(wqk_sb[:], wqk_d[:])
            wv_sb = wpool.tile([128, 2, 2, C], fp8, tag="wv")
            nc.sync.dma_start(wv_sb[:], wv_d[:])
            wp_sb = wpool.tile([128, 2, 2, C], fp8, tag="wp")
            nc.sync.dma_start(wp_sb[:], wp_d[:])

            qkb_sb = wpool.tile([128, 2 * C // 128], f32, tag="qkb")
            nc.sync.dma_start(qkb_sb[:], qkb_d[:])
            pb_sb = wpool.tile([128, CT], f32, tag="pb")
            nc.sync.dma_start(pb_sb[:], pb_d[:])
            gam_sb = wpool.tile([128, CT], f32, tag="gam")
            nc.sync.dma_start(gam_sb[:], gam_d[:])
            bet_sb = wpool.tile([128, CT], f32, tag="bet")
            nc.sync.dma_start(bet_sb[:], bet_d[:])
            # vb broadcast to all partitions, twice along free (nt-pair stt)
            vb_bc2 = wpool.tile([128, 2, C], f32, tag="vbbc")
            nc.sync.dma_start(
                vb_bc2[:],
                bass.AP(tensor=vb_d.tensor, offset=vb_d.offset,
                        ap=[[0, 128], [0, 2], [1, C]]))

            sel = wpool.tile([128, 2], f32, tag="sel")
            nc.vector.memset(sel[0:64, 0:1], 1.0 / GS)
            nc.vector.memset(sel[64:128, 0:1], 0.0)
            nc.vector.memset(sel[0:64, 1:2], 0.0)
            nc.vector.memset(sel[64:128, 1:2], 1.0 / GS)
            # [128, 2, 16]: k-subtile plane step must be 16B-aligned for
            # DoubleRow ldweights; only column 0 is used.
            ones8 = wpool.tile([128, 2, 16], fp8, tag="ones8")
            nc.vector.memset(ones8[:], OS)
            eps_t = wpool.tile([2, 1], f32, tag="eps")
            nc.vector.memset(eps_t[:], EPS)
            eb_t = wpool.tile([128, 1], f32, tag="eb")
            nc.vector.memset(eb_t[:], EB)

            def stage_gn(img):
                """GroupNorm stats on DVE, apply on GpSimd -> xn fp8."""
                xt = xts[img]
                stats2 = spool.tile([128, 2 * CT], f32, tag="stats2",
                                    name=f"stats2_{img}")
                for ct in range(CT):
                    st = spool.tile([128, 2, 6], f32, tag="bnst", name="st")
                    nc.vector.bn_stats(st[:, 0, :], xt[:, ct, 0:512])
                    nc.vector.bn_stats(st[:, 1, :], xt[:, ct, 512:1024])
                    mv = spool.tile([128, 2], f32, tag="mv", name="mv")
                    nc.vector.bn_aggr(mv[:], st[:])
                    nc.vector.tensor_copy(stats2[:, 2 * ct:2 * ct + 1], mv[:, 0:1])
                    nc.vector.tensor_mul(
                        stats2[:, 2 * ct + 1:2 * ct + 2], mv[:, 0:1], mv[:, 0:1])
                    nc.vector.tensor_add(
                        stats2[:, 2 * ct + 1:2 * ct + 2],
                        stats2[:, 2 * ct + 1:2 * ct + 2], mv[:, 1:2])
                psg_t = mmps.tile([128, 2, 512], f32, tag="mm", name="psg")
                psg = psg_t[0:2, 0, 0:2 * CT]
                nc.tensor.matmul(psg, sel[:], stats2[:], start=True, stop=True)
                gs = spool.tile([2, 2 * CT], f32, tag="gs", name="gs")
                nc.scalar.activation(gs[:], psg, AF.Copy)
                gs3 = gs[:].rearrange("p (t s) -> p t s", s=2)
                tmp = spool.tile([2, CT], f32, tag="gtmp", name="tmp")
                nc.vector.tensor_mul(tmp[:], gs3[:, :, 0], gs3[:, :, 0])
                var_g = spool.tile([2, CT], f32, tag="gvar", name="var_g")
                nc.vector.tensor_sub(var_g[:], gs3[:, :, 1], tmp[:])
                sd = spool.tile([2, CT], f32, tag="gsd", name="sd")
                nc.scalar.activation(sd[:], var_g[:], AF.Sqrt, bias=eps_t[:])
                rstd_g = spool.tile([2, CT], f32, tag="grstd", name="rstd_g")
                rscr = spool.tile([2, CT], f32, tag="grscr", name="rscr")
                nc.vector.reciprocal_approx_accurate(rstd_g[:], sd[:], rscr[:])

                xnt = xnpool.tile([128, 2, 2, N], fp8, tag="xn", name=f"xn{img}")
                for ct in range(CT):
                    mu_ch = chpool.tile([128, 1], f32, tag="much", name="mu_ch")
                    sg = gs[:, 2 * ct:2 * ct + 1]
                    nc.sync.dma_start(
                        mu_ch[:],
                        bass.AP(tensor=sg.tensor, offset=sg.offset,
                                ap=[[sg.ap[0][0], 2], [0, GS]]))
                    rs_ch = chpool.tile([128, 1], f32, tag="rsch", name="rs_ch")
                    sg = rstd_g[:, ct:ct + 1]
                    nc.sync.dma_start(
                        rs_ch[:],
                        bass.AP(tensor=sg.tensor, offset=sg.offset,
                                ap=[[sg.ap[0][0], 2], [0, GS]]))
                    a_ch = chpool.tile([128, 1], f32, tag="ach", name="a_ch")
                    nc.vector.tensor_mul(a_ch[:], rs_ch[:], gam_sb[:, ct:ct + 1])
                    b_ch = chpool.tile([128, 1], f32, tag="bch", name="b_ch")
                    nc.vector.tensor_mul(b_ch[:], mu_ch[:], a_ch[:])
                    nc.vector.tensor_sub(b_ch[:], bet_sb[:, ct:ct + 1], b_ch[:])
                    nc.gpsimd.tensor_scalar(
                        out=xnt[:, ct // 2, ct % 2, :], in0=xt[:, ct, :],
                        scalar1=a_ch[:], scalar2=b_ch[:],
                        op0=OP.mult, op1=OP.add)
                return xnt

            def stage_qkv(img, xnt):
                """q,k (bf16, channel-major) + v (fp8, token-major, planes)."""
                qk = qkpool.tile([128, 2 * NH, N], bf16, tag="qk", name=f"qk{img}")
                # mt order pairs q_h with k_h so attn can start early
                for mt in (0, 4, 1, 5, 2, 6, 3, 7):
                    ps = mmps.tile([128, 2, 512], f32, tag="mm", name=f"qkps{mt}")
                    for kp in range(2):
                        for ch in range(2):
                            nc.tensor.matmul(
                                ps[:, ch, :],
                                wqk_sb[:, kp, :, mt * 128:(mt + 1) * 128],
                                xnt[:, kp, :, ch * 512:(ch + 1) * 512],
                                start=(kp == 0), stop=(kp == 1), perf_mode=DR)
                    if mt < 4:  # q: Scalar evac (unscale + bias + bf16 cast)
                        nc.scalar.activation(
                            qk[:, mt, :], ps[:, :, :], AF.Identity,
                            bias=qkb_sb[:, mt:mt + 1], scale=1.0 / WS)
                    else:       # k: DVE evac
                        nc.vector.tensor_scalar(
                            out=qk[:, mt, :], in0=ps[:, :, :],
                            scalar1=1.0 / WS, scalar2=qkb_sb[:, mt:mt + 1],
                            op0=OP.mult, op1=OP.add)

                vt = vtpool.tile([128, NT // 2, 2, C], fp8, tag="vt",
                                 name=f"vt{img}")
                for j in range(NT // 2):
                    ps = mmps.tile([128, 2, 512], f32, tag="mm", name=f"vps{j}")
                    for s in range(2):
                        nt = 2 * j + s
                        for kp in range(2):
                            nc.tensor.matmul(
                                ps[:, s, :],
                                xnt[:, kp, :, nt * 128:(nt + 1) * 128],
                                wv_sb[:, kp, :, :],
                                start=(kp == 0), stop=(kp == 1), perf_mode=DR)
                    nc.vector.scalar_tensor_tensor(
                        out=vt[:, j, :, :], in0=ps[:, :, :], scalar=1.0 / WS,
                        in1=vb_bc2[:, :, :], op0=OP.mult, op1=OP.add)
                return qk, vt

            def head_S(img, h, qk):
                """S^T = K^T Q (bf16) -> exp -> pt fp8 with mt-pair planes."""
                pt = ptpool.tile([128, NT // 2, 2, N], fp8, tag="pt",
                                 name=f"pt{img}_{h}")
                for mt in range(NT):
                    ps = mmps.tile([128, 2, 512], f32, tag="mm",
                                   name=f"sps{mt}")
                    for ch in range(2):
                        nc.tensor.matmul(
                            ps[:, ch, :],
                            qk[:, NH + h, mt * 128:(mt + 1) * 128],
                            qk[:, h, ch * 512:(ch + 1) * 512],
                            start=True, stop=True)
                    nc.scalar.activation(
                        pt[:, mt // 2, mt % 2, :], ps[:, :, :], AF.Exp,
                        bias=eb_t[:], scale=SCALE)
                return pt

            def head_RPV(img, h, pt, vt, ot):
                """rowsum + PV (both fp8 DoubleRow), then normalize."""
                pv = pvps.tile([128, 2, 512], f32, tag="pv", name="pv")
                rs = rsps.tile([1, 2, 512], f32, tag="rs", name="rs")
                for mp in range(NT // 2):
                    for ch in range(2):
                        nc.tensor.matmul(
                            rs[:, ch, :],
                            ones8[:, :, 0:1],
                            pt[:, mp, :, ch * 512:(ch + 1) * 512],
                            start=(mp == 0), stop=(mp == NT // 2 - 1),
                            perf_mode=DR)
                    for ch in range(2):
                        nc.tensor.matmul(
                            pv[:, ch, :],
                            vt[:, mp, :, h * 128:(h + 1) * 128],
                            pt[:, mp, :, ch * 512:(ch + 1) * 512],
                            start=(mp == 0), stop=(mp == NT // 2 - 1),
                            perf_mode=DR)
                rinv = rpool.tile([1, N], f32, tag="rinv", name="rinv", bufs=2)
                nc.vector.reciprocal_approx_fast(rinv[:], rs[0:1, :, :])
                rb = rpool.tile([128, N], f32, tag="rb", name="rb")
                nc.gpsimd.partition_broadcast(rb[:], rinv[:], channels=128)
                # ot = pv * rb  (= 16 * attnout, good fp8 range)
                nc.vector.tensor_mul(ot[:, h // 2, h % 2, :], pv[:, :, :], rb[:])

            def stage_attn(img, qk, vt, after_head=None):
                ot = otpool.tile([128, 2, 2, N], fp8, tag="ot", name=f"ot{img}")
                pts = {}
                pts[0] = head_S(img, 0, qk)
                for h in range(1, NH):
                    pts[h] = head_S(img, h, qk)
                    head_RPV(img, h - 1, pts[h - 1], vt, ot)
                    if after_head is not None:
                        after_head(h - 1)
                head_RPV(img, NH - 1, pts[NH - 1], vt, ot)
                if after_head is not None:
                    after_head(NH - 1)
                return ot

            def emit_proj(img, ot, t):
                ps = mmps.tile([128, 2, 512], f32, tag="mm", name=f"pps{t}")
                for hp in range(2):
                    for ch in range(2):
                        nc.tensor.matmul(
                            ps[:, ch, :],
                            wp_sb[:, hp, :, t * 128:(t + 1) * 128],
                            ot[:, hp, :, ch * 512:(ch + 1) * 512],
                            start=(hp == 0), stop=(hp == 1), perf_mode=DR)
                tmp = tpool.tile([128, N], f32, tag="tmp", name=f"tmp{img}_{t}")
                nc.vector.tensor_scalar(
                    out=tmp[:], in0=ps[:, :, :],
                    scalar1=1.0 / (WS * 16.0), scalar2=pb_sb[:, t:t + 1],
                    op0=OP.mult, op1=OP.add)
                outt = outpool.tile([128, N], f32, tag="outt",
                                    name=f"o{img}_{t}")
                nc.gpsimd.tensor_add(outt[:], tmp[:], xts[img][:, t, :])
                nc.sync.dma_start(
                    out_d[img, t * 128:(t + 1) * 128, :], outt[:])

            # ---- software pipeline over the two images ----
            xn0 = stage_gn(0)
            qk0, vt0 = stage_qkv(0, xn0)
            xn1 = stage_gn(1)
            ot0 = stage_attn(0, qk0, vt0)
            qk1, vt1 = stage_qkv(1, xn1)
            ot1 = stage_attn(1, qk1, vt1,
                             after_head=lambda t: emit_proj(0, ot0, t))
            for t in range(CT):
                emit_proj(1, ot1, t)

    nc.compile()
    return nc


_NC_CACHE = None


def _get_nc():
    global _NC_CACHE
    if _NC_CACHE is None:
        _NC_CACHE = build_program()
    return _NC_CACHE


def _host_prep(x, norm_gamma, norm_beta, qkv_w, qkv_b, proj_w, proj_b):
    import ml_dtypes
    f8 = ml_dtypes.float8_e4m3

    def pack_w(wT):  # [c=512, o] -> [128, 2, 2, o] fp8, prescaled
        o = wT.shape[1]
        return np.ascontiguousarray(
            (wT.reshape(2, 2, 128, o) * WS).transpose(2, 0, 1, 3)
        ).astype(f8)

    qkv_w = np.asarray(qkv_w, dtype=np.float32)
    proj_w = np.asarray(proj_w, dtype=np.float32)
    qkv_b = np.asarray(qkv_b, dtype=np.float32)
    common = {
        "wqk": pack_w(qkv_w[:2 * C].T),
        "wv": pack_w(qkv_w[2 * C:].T),
        "wp": pack_w(proj_w.T),
        "qkb": np.ascontiguousarray(qkv_b[:2 * C].reshape(-1, 128).T),
        "vb": np.ascontiguousarray(qkv_b[2 * C:]),
        "pb": np.ascontiguousarray(
            np.asarray(proj_b, dtype=np.float32).reshape(CT, 128).T),
        "gamma": np.ascontiguousarray(
            np.asarray(norm_gamma, dtype=np.float32).reshape(CT, 128).T),
        "beta": np.ascontiguousarray(
            np.asarray(norm_beta, dtype=np.float32).reshape(CT, 128).T),
    }
    xr = np.ascontiguousarray(np.asarray(x, dtype=np.float32).reshape(B, C, N))
    in_maps = []
    for c in range(N_CORES):
        m = dict(common)
        m["x"] = np.ascontiguousarray(xr[c * B_LOC:(c + 1) * B_LOC])
        in_maps.append(m)
    return in_maps


def run(inputs, trace=False):
    nc = _get_nc()
    in_maps = _host_prep(**inputs)
    res = None
    for attempt in range(3):
        try:
            res = run_bass_kernel_spmd(
                nc, in_maps, core_ids=list(range(N_CORES)), trace=trace)
            break
        except Exception:
            # rare transient NRT_EXEC_UNIT_UNRECOVERABLE on a cold device;
            # a re-run on the recovered device succeeds.
            if attempt == 2:
                raise
    parts = [res.results[c]["out"] for c in range(N_CORES)]
    out = np.concatenate(parts, axis=0).reshape(B, C, HH, WW)
    return out.astype(np.float32), res


def kernel(**inputs):
    out, _ = run(inputs, trace=False)
    return out
